# revision 1
# baseline (speedup 1.0000x reference)
"""Trainium2 Bass kernel for a causal single-head attention block.

Problem: y = softmax(mask(Q K^T / sqrt(H))) V with
  x  [B=4, T=4096, C=1024] f32,  Wq/Wk/Wv [C, H=64] f32.

Sharding (8 NeuronCores): data-parallel over B across core pairs;
within a pair, the T dimension is split by interleaved 512-row tiles
(rank r owns global q-tiles {2s+r}) so the causal work is balanced.
Each core computes Q/K/V for its own 2048 rows, the pair exchanges
K^T and V via an AllGather, and each core runs a flash-attention style
kc-outer loop over its own query rows.

The graph is identical on all 8 cores (SPMD); all rank-dependent
causality is delivered via input *data* (a sliding causal mask sheet).

Layout notes:
 - The host pre-transposes x per core to x^T [C, 2048] bf16 so the
   projections can contract over C on the partition dimension without
   any on-chip transpose of x.
 - Projections produce Q^T/K^T/V^T [64, T] directly (H on partitions),
   which is exactly the operand layout the S^T matmul wants.
 - S^T tiles are [128 k, 512 q]; exp has no running max (logits here
   are ~N(0,1), |s| < ~7, so exp is safe in f32) and the row-sum is
   folded into the PV matmul via a ones-column appended to V.
"""

import numpy as np
import ml_dtypes

import concourse.bass as bass
import concourse.bacc as bacc
import concourse.mybir as mybir
from concourse.tile import TileContext
from concourse.bass_utils import run_bass_kernel_spmd

BF16 = mybir.dt.bfloat16
F32 = mybir.dt.float32
bf16 = ml_dtypes.bfloat16

B, T, C, H = 4, 4096, 1024, 64
N_CORES = 8
TOWN = 2048          # rows owned per core
NSLOT = 4            # q-tiles of 512 rows per core
QT512 = 512
KC = 128             # k-chunk rows
NKC = T // KC        # 32 global k-chunks
W_SLOT = [8, 16, 24, 32]   # uniform kc-window per slot
V_FLAT = TOWN * H    # 131072 elements of V shard in the bounce
CC_K = H * TOWN      # K^T shard elements
CC_V = 128 * 1024    # V shard elements
MASK_W = 896 + 512   # causal mask sheet width


def build_bass():
    nc = bacc.Bacc(
        "TRN2",
        target_bir_lowering=False,
        debug=False,
        enable_asserts=False,
        num_devices=N_CORES,
    )

    xT = nc.declare_dram_parameter("xT", [C, TOWN], BF16, isOutput=False)
    wq = nc.declare_dram_parameter("wq", [C, H], BF16, isOutput=False)
    wk = nc.declare_dram_parameter("wk", [C, H], BF16, isOutput=False)
    wv = nc.declare_dram_parameter("wv", [C, H], BF16, isOutput=False)
    ident = nc.declare_dram_parameter("ident", [H, H], BF16, isOutput=False)
    mask = nc.declare_dram_parameter("mask", [128, MASK_W], BF16, isOutput=False)
    out = nc.declare_dram_parameter("out", [H, TOWN], F32, isOutput=True)

    cc_in_k = nc.dram_tensor("cc_in_k", [CC_K], BF16)
    cc_out_k = nc.dram_tensor("cc_out_k", [2 * CC_K], BF16)
    cc_in_v = nc.dram_tensor("cc_in_v", [CC_V], BF16)
    cc_out_v = nc.dram_tensor("cc_out_v", [2 * CC_V], BF16)

    with TileContext(nc) as tc:
        with (
            tc.tile_pool(name="persist", bufs=1) as pp,
            tc.tile_pool(name="work", bufs=3) as wp,
        ):
            # ---- persistent SBUF tensors ----
            xT_sb = pp.tile([128, 8, TOWN], BF16, tag="xT")
            wq_sb = pp.tile([128, 8, H], BF16, tag="wq")
            wk_sb = pp.tile([128, 8, H], BF16, tag="wk")
            wv_sb = pp.tile([128, 8, H], BF16, tag="wv")
            id_sb = pp.tile([H, H], BF16, tag="ident")
            mask_sb = pp.tile([128, MASK_W], BF16, tag="mask")
            qT_sb = pp.tile([H, TOWN], BF16, tag="qT")
            vT_own = pp.tile([H, TOWN], BF16, tag="vTown")
            kstage = pp.tile([H, TOWN], BF16, tag="kstage")
            vstage = pp.tile([128, 1024], BF16, tag="vstage")
            # rank-blocked columns: [rank0 2048 | rank1 2048]
            kT_g = pp.tile([H, T], BF16, tag="kTg")
            # V rank-blocked [128, kcb, 65]; col 64 of each chunk is ones
            vaug = pp.tile([128, NKC, H + 1], BF16, tag="vaug")
            ones_sb = pp.tile([1, H], F32, tag="ones")
            yT_sb = pp.tile([H, TOWN], F32, tag="yT")

            # ---- loads ----
            nc.sync.dma_start(
                out=xT_sb[:], in_=xT[:].rearrange("(cc p) t -> p cc t", p=128)
            )
            nc.sync.dma_start(
                out=wq_sb[:], in_=wq[:].rearrange("(cc p) h -> p cc h", p=128)
            )
            nc.sync.dma_start(
                out=wk_sb[:], in_=wk[:].rearrange("(cc p) h -> p cc h", p=128)
            )
            nc.sync.dma_start(
                out=wv_sb[:], in_=wv[:].rearrange("(cc p) h -> p cc h", p=128)
            )
            nc.sync.dma_start(out=id_sb[:], in_=ident[:])
            nc.sync.dma_start(out=mask_sb[:], in_=mask[:])
            nc.vector.memset(ones_sb[:], 1.0)
            nc.vector.memset(vaug[:, :, H : H + 1], 1.0)

            # ---- projections: Q^T, K^T, V^T for own rows ----
            with tc.tile_pool(name="proj_ps", bufs=2, space="PSUM") as proj_ps:
                def proj(w_sb, dst, sl):
                    ps = proj_ps.tile([H, QT512], F32, tag="proj", name="ps")
                    for cc in range(8):
                        nc.tensor.matmul(
                            ps[:],
                            w_sb[:, cc, :],
                            xT_sb[:, cc, sl],
                            start=(cc == 0),
                            stop=(cc == 7),
                        )
                    nc.vector.tensor_copy(dst, ps[:])

                # K first so its exchange can start ASAP
                for tt in range(NSLOT):
                    sl = slice(tt * QT512, (tt + 1) * QT512)
                    proj(wk_sb, kstage[:, sl], sl)
                nc.gpsimd.dma_start(
                    out=cc_in_k[:].rearrange("(p t) -> p t", p=H), in_=kstage[:]
                )
                nc.gpsimd.collective_compute(
                    "AllGather",
                    mybir.AluOpType.bypass,
                    replica_groups=[[2 * i, 2 * i + 1] for i in range(N_CORES // 2)],
                    ins=[cc_in_k[:]],
                    outs=[cc_out_k[:]],
                )

                for tt in range(NSLOT):
                    sl = slice(tt * QT512, (tt + 1) * QT512)
                    proj(wv_sb, vT_own[:, sl], sl)
                for tcn in range(16):
                    pst = proj_ps.tile([128, H], BF16, tag="vt")
                    nc.tensor.transpose(
                        pst[:], vT_own[:, tcn * 128 : (tcn + 1) * 128], id_sb[:]
                    )
                    nc.vector.tensor_copy(vstage[:, tcn * H : (tcn + 1) * H], pst[:])
                nc.gpsimd.dma_start(
                    out=cc_in_v[:].rearrange("(p c) -> p c", p=128), in_=vstage[:]
                )
                nc.gpsimd.collective_compute(
                    "AllGather",
                    mybir.AluOpType.bypass,
                    replica_groups=[[2 * i, 2 * i + 1] for i in range(N_CORES // 2)],
                    ins=[cc_in_v[:]],
                    outs=[cc_out_v[:]],
                )

                # Q projection overlaps the collectives
                for tt in range(NSLOT):
                    sl = slice(tt * QT512, (tt + 1) * QT512)
                    proj(wq_sb, qT_sb[:, sl], sl)

            # ---- readback of gathered K^T and V ----
            nc.gpsimd.dma_start(
                out=kT_g[:].rearrange("p (gp t) -> p gp t", gp=2),
                in_=cc_out_k[:].rearrange("(gp p t) -> p gp t", gp=2, p=H),
            )
            blkv = cc_out_v[:].rearrange("(gp p c) -> gp p c", gp=2, p=128)
            for gp in range(2):
                nc.gpsimd.dma_start(
                    out=vaug[:, gp * 16 : (gp + 1) * 16, 0:H],
                    in_=blkv[gp].rearrange("p (tc h) -> p tc h", h=H),
                )

            # ---- attention: kc-outer flash loop ----
            with (
                tc.tile_pool(name="swide", bufs=2, space="PSUM") as sp,
                tc.tile_pool(name="yacc", bufs=1, space="PSUM") as yp,
            ):
                y_acc = [
                    yp.tile([128, QT512], F32, tag=f"y{s}", name=f"y_acc{s}")
                    for s in range(NSLOT)
                ]

                for kc in range(NKC):
                    g = kc // 4
                    kcol = (g % 2) * 2048 + (g // 2) * QT512 + (kc % 4) * KC
                    kcb = (g % 2) * 16 + (g // 2) * 4 + kc % 4
                    smin = kc // 8
                    slots = list(range(smin, NSLOT))
                    for gi in range(0, len(slots), 2):
                        grp = slots[gi : gi + 2]
                        fd = QT512 * len(grp)
                        sw = sp.tile([128, 1024], F32, tag="swide")
                        for i, s in enumerate(grp):
                            nc.tensor.matmul(
                                sw[:, i * QT512 : (i + 1) * QT512],
                                kT_g[:, kcol : kcol + KC],
                                qT_sb[:, s * QT512 : (s + 1) * QT512],
                                start=True,
                                stop=True,
                            )
                        pt = wp.tile([128, 1024], BF16, tag="pt")
                        nc.scalar.activation(
                            pt[:, 0:fd],
                            sw[:, 0:fd],
                            mybir.ActivationFunctionType.Exp,
                            scale=float(H) ** -0.5,
                        )
                        for i, s in enumerate(grp):
                            psl = pt[:, i * QT512 : (i + 1) * QT512]
                            j = kc - 8 * s
                            if 0 <= j < 8:
                                o = (7 - j) * 128
                                nc.vector.tensor_mul(
                                    psl, psl, mask_sb[:, o : o + QT512]
                                )
                            nc.tensor.matmul(
                                y_acc[s][0 : H + 1, :],
                                vaug[:, kcb, :],
                                psl,
                                start=(kc == 0),
                                stop=(kc == W_SLOT[s] - 1),
                            )

                # ---- normalize and write out ----
                lsum = wp.tile([1, NSLOT * QT512], F32, tag="lsum")
                for s in range(NSLOT):
                    nc.vector.tensor_copy(
                        lsum[0:1, s * QT512 : (s + 1) * QT512], y_acc[s][H : H + 1, :]
                    )
                rec = wp.tile([1, NSLOT * QT512], F32, tag="rec")
                nc.vector.reciprocal(rec[:], lsum[:])
                for s in range(NSLOT):
                    sl = slice(s * QT512, (s + 1) * QT512)
                    bc = sp.tile([H, QT512], F32, tag="swide")
                    nc.tensor.matmul(
                        bc[:],
                        ones_sb[:],
                        rec[0:1, s * QT512 : (s + 1) * QT512],
                        start=True,
                        stop=True,
                    )
                    bc_sb = wp.tile([H, QT512], F32, tag="bcsb")
                    nc.vector.tensor_copy(bc_sb[:], bc[:])
                    nc.vector.tensor_mul(yT_sb[:, sl], y_acc[s][0:H, :], bc_sb[:])

            nc.sync.dma_start(out=out[:], in_=yT_sb[:])

    nc.compile()
    return nc


_NC_CACHE = None


def _get_nc():
    global _NC_CACHE
    if _NC_CACHE is None:
        _NC_CACHE = build_bass()
    return _NC_CACHE


def _make_in_maps(x, Wq, Wk, Wv):
    ident = np.eye(H, dtype=bf16)
    wq16, wk16, wv16 = (w.astype(bf16) for w in (Wq, Wk, Wv))
    p_idx = np.arange(128)[:, None]
    x_idx = np.arange(MASK_W)[None, :]
    masks = [
        (p_idx <= x_idx - off).astype(bf16) for off in (896, 384)
    ]  # rank 0 owns even tiles, rank 1 odd tiles
    in_maps = []
    for c in range(N_CORES):
        b, r = divmod(c, 2)
        rows = np.concatenate(
            [x[b, (2 * s + r) * QT512 : (2 * s + r + 1) * QT512] for s in range(NSLOT)]
        )
        xT_c = np.ascontiguousarray(rows.T).astype(bf16)
        in_maps.append(
            {
                "xT": xT_c,
                "wq": wq16,
                "wk": wk16,
                "wv": wv16,
                "ident": ident,
                "mask": masks[r],
            }
        )
    return in_maps


def _assemble(results):
    y = np.empty((B, T, H), dtype=np.float32)
    for c in range(N_CORES):
        b, r = divmod(c, 2)
        yt = np.asarray(results[c]["out"], dtype=np.float32).T  # [2048, 64]
        for s in range(NSLOT):
            g = 2 * s + r
            y[b, g * QT512 : (g + 1) * QT512] = yt[s * QT512 : (s + 1) * QT512]
    return y


def run(x, Wq, Wk, Wv, trace=False):
    nc = _get_nc()
    in_maps = _make_in_maps(
        np.asarray(x, np.float32),
        np.asarray(Wq, np.float32),
        np.asarray(Wk, np.float32),
        np.asarray(Wv, np.float32),
    )
    res = run_bass_kernel_spmd(nc, in_maps, core_ids=list(range(N_CORES)), trace=trace)
    return _assemble(res.results), res


def kernel(x, Wq, Wk, Wv):
    y, _ = run(x, Wq, Wk, Wv)
    return y



# revision 8
# speedup vs baseline: 1.2722x; 1.2722x over previous
"""Trainium2 Bass kernel for a causal single-head attention block.

Problem: y = softmax(mask(Q K^T / sqrt(H))) V with
  x  [B=4, T=4096, C=1024] f32,  Wq/Wk/Wv [C, H=64] f32.

Sharding (8 NeuronCores): data-parallel over B across core pairs;
within a pair, T is split by interleaved 512-row tiles (rank r owns
global q-tiles {2s+r}).  Each core projects K/V for its own 2048 rows,
the pair exchanges them via AllGather, and each core runs a
flash-attention style pair-of-kc outer loop over its own query rows.
The graph is identical on all 8 cores (SPMD); all rank-dependent
causality is delivered via input *data* (per-rank mask sheets).

v2 design notes (vs the first working version):
 - K and V are projected together with a packed [Wk|Wv] stationary so
   both collectives can start as early as possible; Q is projected as
   [Wq|Wq] so both partition halves of the PE array get a copy of Q^T
   with a single PSUM->SBUF copy (needed for h0/h1 row-group pairing).
 - S^T matmuls are issued in pairs on disjoint PE row groups
   (contraction=64 at partitions 0:64 and 64:128) so consecutive
   LDWEIGHTS/MATMULs overlap in the array.
 - exp runs on ACT over [128,1024] PSUM tiles (scale folded in); the
   causal mask is a bf16 multiply on DVE for the last 4 pairs of each
   slot's window only.
 - The row-sum is folded into PV via a ones-column (65-wide V tiles);
   normalization uses a DMA partition-reshape + one DVE
   reciprocal_approx_fast over [128,16] instead of a 15us single-lane
   reciprocal.
 - xT streams in per-slot so projections start after ~1MiB landed.
"""

import numpy as np
import ml_dtypes

import concourse.bass as bass
import concourse.bacc as bacc
import concourse.mybir as mybir
from concourse.tile import TileContext
from concourse.bass_utils import run_bass_kernel_spmd

BF16 = mybir.dt.bfloat16
F32 = mybir.dt.float32
bf16 = ml_dtypes.bfloat16

B, T, C, H = 4, 4096, 1024, 64
N_CORES = 8
TOWN = 2048          # rows owned per core
NSLOT = 4            # q-tiles of 512 rows per core
QT = 512
NKC = 32             # global 128-row k-chunks
NPAIR = 16           # global 256-row k-pair chunks
CC_K = H * TOWN      # K^T shard elements
CC_V = 128 * 1024    # V shard elements (t-layout)


def build_bass(dbg=False):
    nc = bacc.Bacc(
        "TRN2",
        target_bir_lowering=False,
        debug=False,
        enable_asserts=False,
        num_devices=N_CORES,
    )

    if dbg:
        d_kv = nc.declare_dram_parameter("d_kv", [128, TOWN], BF16, isOutput=True)
        d_q0 = nc.declare_dram_parameter("d_q0", [128, QT], BF16, isOutput=True)
        d_kT2 = nc.declare_dram_parameter("d_kT2", [128, T // 2], BF16, isOutput=True)
        d_vaug = nc.declare_dram_parameter(
            "d_vaug", [128, NKC * (H + 1)], BF16, isOutput=True
        )
        d_lrow = nc.declare_dram_parameter(
            "d_lrow", [1, NSLOT * QT], F32, isOutput=True
        )
        d_rrow = nc.declare_dram_parameter(
            "d_rrow", [1, NSLOT * QT], F32, isOutput=True
        )

    xT = nc.declare_dram_parameter("xT", [C, TOWN], BF16, isOutput=False)
    wkv = nc.declare_dram_parameter("wkv", [C, 128], BF16, isOutput=False)
    wqq = nc.declare_dram_parameter("wqq", [C, 128], BF16, isOutput=False)
    ident = nc.declare_dram_parameter("ident", [128, H], BF16, isOutput=False)
    mask = nc.declare_dram_parameter("mask", [128, 4 * 1024], BF16, isOutput=False)
    out = nc.declare_dram_parameter("out", [H, TOWN], F32, isOutput=True)

    cc_in_k = nc.dram_tensor("cc_in_k", [CC_K], BF16)
    cc_out_k = nc.dram_tensor("cc_out_k", [2 * CC_K], BF16)
    cc_in_v = nc.dram_tensor("cc_in_v", [CC_V], BF16)
    cc_out_v = nc.dram_tensor("cc_out_v", [2 * CC_V], BF16)
    lsc = nc.dram_tensor("lsc", [NSLOT * QT], F32)
    rsc = nc.dram_tensor("rsc", [NSLOT * QT], F32)
    groups = [[2 * i, 2 * i + 1] for i in range(N_CORES // 2)]

    with TileContext(nc) as tc:
        with (
            tc.tile_pool(name="persist", bufs=1) as pp,
            tc.tile_pool(name="work", bufs=3) as wp,
        ):
            # ---- persistent SBUF tensors ----
            xT_sb = pp.tile([128, 8, TOWN], BF16, tag="xT")
            wkv_sb = pp.tile([128, 8, 128], BF16, tag="wkv")
            wqq_sb = pp.tile([128, 8, 128], BF16, tag="wqq")
            id_sb = pp.tile([128, H], BF16, tag="ident")
            mask_sb = pp.tile([128, 4 * 1024], BF16, tag="mask")
            # K^T (rows 0:64) and V^T (rows 64:128) staging for own rows
            kv_stage = pp.tile([128, TOWN], BF16, tag="kvstage")
            vstage = pp.tile([128, 1024], BF16, tag="vstage")
            # Q^T duplicated on both partition halves, per slot
            qT2 = [
                pp.tile([128, QT], BF16, tag=f"q{s}", name=f"qT2_{s}")
                for s in range(NSLOT)
            ]
            # K^T pair layout: pair p cols p*128..(p+1)*128; chunk 2p at
            # partitions 0:64, chunk 2p+1 at partitions 64:128
            kT2 = pp.tile([128, T // 2], BF16, tag="kT2")
            # V t-layout chunks + ones column (col 64 of each chunk)
            vaug = pp.tile([128, NKC, H + 1], BF16, tag="vaug")
            ones_sb = pp.tile([1, H], F32, tag="ones")
            lrow = pp.tile([1, NSLOT * QT], F32, tag="lrow")
            lsum_t = pp.tile([128, 16], F32, tag="lsumt")
            rec_t = pp.tile([128, 16], F32, tag="rect")
            rec_row = pp.tile([1, NSLOT * QT], F32, tag="recrow")

            # ---- loads ----
            nc.sync.dma_start(
                out=wkv_sb[:], in_=wkv[:].rearrange("(cc p) m -> p cc m", p=128)
            )
            nc.sync.dma_start(
                out=wqq_sb[:], in_=wqq[:].rearrange("(cc p) m -> p cc m", p=128)
            )
            nc.sync.dma_start(out=id_sb[:], in_=ident[:])
            nc.gpsimd.dma_start(out=mask_sb[:], in_=mask[:])
            nc.vector.memset(ones_sb[:], 1.0)
            nc.vector.memset(vaug[:, :, H : H + 1], 1.0)
            # xT streamed per slot (1 MiB each), alternating trigger queues
            xq = [nc.sync, nc.gpsimd, nc.sync, nc.gpsimd]
            for s in range(NSLOT):
                sl = slice(s * QT, (s + 1) * QT)
                xq[s].dma_start(
                    out=xT_sb[:, :, sl],
                    in_=xT[:, sl].rearrange("(cc p) t -> p cc t", p=128),
                )

            # ---- projections ----
            with (
                tc.tile_pool(name="proj_ps", bufs=2, space="PSUM") as proj_ps,
                tc.tile_pool(name="vt_ps", bufs=2, space="PSUM") as vt_ps,
            ):
                # pass 1: K^T | V^T for own rows (packed stationary)
                for s in range(NSLOT):
                    sl = slice(s * QT, (s + 1) * QT)
                    ps = proj_ps.tile([128, QT], F32, tag="proj")
                    for cc in range(8):
                        nc.tensor.matmul(
                            ps[:],
                            wkv_sb[:, cc, :],
                            xT_sb[:, cc, sl],
                            start=(cc == 0),
                            stop=(cc == 7),
                        )
                    nc.vector.tensor_copy(kv_stage[:, sl], ps[:])
                nc.gpsimd.dma_start(
                    out=cc_in_k[:].rearrange("(p t) -> p t", p=H),
                    in_=kv_stage[0:H, :],
                )
                nc.gpsimd.collective_compute(
                    "AllGather",
                    mybir.AluOpType.bypass,
                    replica_groups=groups,
                    ins=[cc_in_k[:]],
                    outs=[cc_out_k[:]],
                )

                # V^T -> V (t-layout) via PE transposes on partitions 64:128
                for tcn in range(16):
                    pst = vt_ps.tile([128, H], BF16, tag="vt")
                    nc.tensor.transpose(
                        pst[:],
                        kv_stage[64:128, tcn * 128 : (tcn + 1) * 128],
                        id_sb[64:128, :],
                    )
                    nc.vector.tensor_copy(vstage[:, tcn * H : (tcn + 1) * H], pst[:])
                nc.gpsimd.dma_start(
                    out=cc_in_v[:].rearrange("(p c) -> p c", p=128), in_=vstage[:]
                )
                nc.gpsimd.collective_compute(
                    "AllGather",
                    mybir.AluOpType.bypass,
                    replica_groups=groups,
                    ins=[cc_in_v[:]],
                    outs=[cc_out_v[:]],
                )

                # pass 2: Q^T duplicated to both halves ([Wq|Wq] stationary)
                for s in range(NSLOT):
                    sl = slice(s * QT, (s + 1) * QT)
                    ps = proj_ps.tile([128, QT], F32, tag="proj")
                    for cc in range(8):
                        nc.tensor.matmul(
                            ps[:],
                            wqq_sb[:, cc, :],
                            xT_sb[:, cc, sl],
                            start=(cc == 0),
                            stop=(cc == 7),
                        )
                    nc.vector.tensor_copy(qT2[s][:], ps[:])

            # ---- readback of gathered K^T and V into compute layouts ----
            # shard gp holds tiles {2s+gp}; tile g = 2s+gp -> pairs 2g, 2g+1
            ck = cc_out_k[:].rearrange("(gp h sc) -> gp h sc", gp=2, h=H)
            cv = cc_out_v[:].rearrange("(gp p sc) -> gp p sc", gp=2, p=128)
            for gp in range(2):
                for s in range(NSLOT):
                    g = 2 * s + gp
                    ck_s = ck[gp, :, s * QT : (s + 1) * QT].rearrange(
                        "h (chalf hh kk) -> h chalf hh kk", chalf=2, hh=2
                    )
                    for hh in range(2):
                        nc.sync.dma_start(
                            out=kT2[
                                hh * 64 : (hh + 1) * 64,
                                2 * g * 128 : (2 * g + 2) * 128,
                            ].rearrange("h (chalf kk) -> h chalf kk", chalf=2),
                            in_=ck_s[:, :, hh, :],
                        )
                    nc.gpsimd.dma_start(
                        out=vaug[:, 4 * g : 4 * g + 4, 0:H],
                        in_=cv[gp, :, s * 256 : (s + 1) * 256].rearrange(
                            "p (c h) -> p c h", h=H
                        ),
                    )

            # ---- attention: pair-of-kc outer flash loop ----
            with (
                tc.tile_pool(name="swide", bufs=2, space="PSUM") as sp,
                tc.tile_pool(name="yacc", bufs=1, space="PSUM") as yp,
            ):
                y_acc = [
                    yp.tile([128, QT], F32, tag=f"y{s}", name=f"y_acc{s}")
                    for s in range(NSLOT)
                ]

                def norm_slot(s):
                    # lsum row (PSUM partition 64) -> SBUF row
                    nc.vector.tensor_copy(
                        lrow[0:1, s * QT : (s + 1) * QT], y_acc[s][H : H + 1, :]
                    )

                for p in range(NPAIR):
                    smin = max(0, (p - 3 + 3) // 4)  # slots with p <= 4s+3
                    for s in range(smin, NSLOT):
                        if p > 4 * s + 3:
                            continue
                        sw = sp.tile([128, 1024], F32, tag="swide")
                        nc.tensor.matmul(
                            sw[:, 0:QT],
                            kT2[0:64, p * 128 : (p + 1) * 128],
                            qT2[s][0:64, :],
                            start=True,
                            stop=True,
                        )
                        nc.tensor.matmul(
                            sw[:, QT:1024],
                            kT2[64:128, p * 128 : (p + 1) * 128],
                            qT2[s][64:128, :],
                            start=True,
                            stop=True,
                        )
                        pt = wp.tile([128, 1024], BF16, tag="pt")
                        nc.scalar.activation(
                            pt[:],
                            sw[:],
                            mybir.ActivationFunctionType.Exp,
                            scale=float(H) ** -0.5,
                        )
                        pp_idx = p - 4 * s
                        if pp_idx >= 0:
                            nc.vector.tensor_mul(
                                pt[:],
                                pt[:],
                                mask_sb[:, pp_idx * 1024 : (pp_idx + 1) * 1024],
                            )
                        for half in range(2):
                            kc = 2 * p + half
                            nc.tensor.matmul(
                                y_acc[s][0 : H + 1, :],
                                vaug[:, kc, :],
                                pt[:, half * QT : (half + 1) * QT],
                                start=(p == 0 and half == 0),
                                stop=(p == 4 * s + 3 and half == 1),
                            )
                        if p == 4 * s + 3:
                            norm_slot(s)

                # ---- normalize and write out ----
                # partition reshape must bounce through DRAM (linear space)
                nc.sync.dma_start(
                    out=lsc[:].rearrange("(one f) -> one f", one=1), in_=lrow[:]
                )
                nc.sync.dma_start(
                    out=lsum_t[:], in_=lsc[:].rearrange("(p f) -> p f", p=128)
                )
                nc.vector.reciprocal_approx_fast(rec_t[:], lsum_t[:])
                nc.sync.dma_start(
                    out=rsc[:].rearrange("(p f) -> p f", p=128), in_=rec_t[:]
                )
                nc.sync.dma_start(
                    out=rec_row[:], in_=rsc[:].rearrange("(one f) -> one f", one=1)
                )
                if dbg:
                    nc.sync.dma_start(out=d_kv[:], in_=kv_stage[:])
                    nc.sync.dma_start(out=d_q0[:], in_=qT2[0][:])
                    nc.sync.dma_start(out=d_kT2[:], in_=kT2[:])
                    nc.sync.dma_start(
                        out=d_vaug[:],
                        in_=vaug[:].rearrange("p a b -> p (a b)"),
                    )
                    nc.sync.dma_start(out=d_lrow[:], in_=lrow[:])
                    nc.sync.dma_start(out=d_rrow[:], in_=rec_row[:])
                for s in range(NSLOT):
                    sl = slice(s * QT, (s + 1) * QT)
                    bc = sp.tile([H, QT], F32, tag="swide")
                    nc.tensor.matmul(
                        bc[:],
                        ones_sb[:],
                        rec_row[0:1, sl],
                        start=True,
                        stop=True,
                    )
                    bc_sb = wp.tile([H, QT], F32, tag="bcsb")
                    nc.vector.tensor_copy(bc_sb[:], bc[:])
                    y_sb = wp.tile([H, QT], F32, tag="ysb")
                    nc.vector.tensor_mul(y_sb[:], y_acc[s][0:H, :], bc_sb[:])
                    nc.sync.dma_start(out=out[:, sl], in_=y_sb[:])

    nc.compile()
    return nc


_NC_CACHE = None


def _get_nc():
    global _NC_CACHE
    if _NC_CACHE is None:
        _NC_CACHE = build_bass()
    return _NC_CACHE


def _make_in_maps(x, Wq, Wk, Wv):
    ident = np.zeros((128, H), dtype=bf16)
    ident[64:128, :] = np.eye(H, dtype=bf16)
    wkv = np.concatenate([Wk, Wv], axis=1).astype(bf16)
    wqq = np.concatenate([Wq, Wq], axis=1).astype(bf16)
    # mask sheets [128, 4*1024]: pair-position pp in 0..3, halves of 512
    # keep iff k <= q: p <= f + 512*r - 256*pp - 128*half
    p_idx = np.arange(128)[:, None]
    masks = []
    for r in range(2):
        m = np.zeros((128, 4, 2, QT), dtype=bf16)
        for ppos in range(4):
            for half in range(2):
                f_idx = np.arange(QT)[None, :]
                keep = p_idx <= f_idx + 512 * r - 256 * ppos - 128 * half
                m[:, ppos, half, :] = keep.astype(bf16)
        masks.append(np.ascontiguousarray(m.reshape(128, 4096)))
    in_maps = []
    for c in range(N_CORES):
        b, r = divmod(c, 2)
        rows = np.concatenate(
            [x[b, (2 * s + r) * QT : (2 * s + r + 1) * QT] for s in range(NSLOT)]
        )
        xT_c = np.ascontiguousarray(rows.T).astype(bf16)
        in_maps.append(
            {
                "xT": xT_c,
                "wkv": wkv,
                "wqq": wqq,
                "ident": ident,
                "mask": masks[r],
            }
        )
    return in_maps


def _assemble(results):
    y = np.empty((B, T, H), dtype=np.float32)
    for c in range(N_CORES):
        b, r = divmod(c, 2)
        yt = np.asarray(results[c]["out"], dtype=np.float32).T  # [2048, 64]
        for s in range(NSLOT):
            g = 2 * s + r
            y[b, g * QT : (g + 1) * QT] = yt[s * QT : (s + 1) * QT]
    return y


def run(x, Wq, Wk, Wv, trace=False):
    nc = _get_nc()
    in_maps = _make_in_maps(
        np.asarray(x, np.float32),
        np.asarray(Wq, np.float32),
        np.asarray(Wk, np.float32),
        np.asarray(Wv, np.float32),
    )
    res = run_bass_kernel_spmd(nc, in_maps, core_ids=list(range(N_CORES)), trace=trace)
    return _assemble(res.results), res


def kernel(x, Wq, Wk, Wv):
    y, _ = run(x, Wq, Wk, Wv)
    return y
